# revision 23
# baseline (speedup 1.0000x reference)
"""Grouped linear (MoE routed GEMM) on 8 Trainium2 NeuronCores.

out[t] = hidden_states[t] @ weight[g(t)] where g(t) is the expert owning
token t (contiguous groups sized by tokens_per_expert).

Strategy (expert-parallel, token-balanced):
  - All group sizes are multiples of 128, so work = 64 row-tiles of 128
    tokens. Each core gets exactly 8 row-tiles (1024 tokens).
  - SPMD requires one program for all cores, so every core runs the same
    static slot pattern [0,0,0,1,1,1,2,2]: 3 weight slots covering 3/3/2
    row-tiles. The host decomposes the per-expert tile counts into
    sixteen 3-tile parts + eight 2-tile parts and assigns (expert ->
    core,slot); each core receives its 1024 tokens pre-transposed and
    its 3 slot weight matrices, packed partition-major in consume order
    (wv0 = per-k [xt_k | w0_k] batches, wv1/wv2 = late weight waves).
  - Device: bf16 matmuls (fp32 PSUM accumulate), K-contiguous chains of
    8 matmuls per [128,512] output tile. Wave-0 batches are rate-limited
    by a depth-5 dependency ladder so tiles arrive progressively and in
    consume order (SDMA round-robins across queues at packet
    granularity; without the ladder every transfer lands at the same
    late time and the PE idles for ~12us); wv1/wv2 issue is gated on
    compute progress. Stores go on the scalar engine's HWDGE ring so
    they never stall behind load waits. A few junk warmup matmuls lift
    the PE HAM clock gate while the first tiles are in flight.

Measured (core 0 NTFF): ~47us vs ~61us for the naive schedule; PE window
is ~94% dense at the 27.3us bf16 roofline for the per-core GEMM, with
~6us bass preamble + ~5us tail (drain + store receipt) fixed overhead.
"""

import os
import numpy as np
import ml_dtypes
from contextlib import ExitStack

import concourse.bass as bass
import concourse.tile as tile
from concourse import bacc, mybir
from concourse.bass_utils import run_bass_kernel_spmd
from concourse.tile import add_dep_helper

T, D, G, NCORES = 8192, 1024, 8, 8
TPC = T // NCORES            # tokens per core
RT = TPC // 128              # row tiles per core (8)
KT = D // 128                # contraction tiles (8)
NSLOTS = 3
PATTERN = (0, 0, 0, 1, 1, 1, 2, 2)   # row-tile -> weight slot
LADDER_DEPTH = int(os.environ.get("K_LADDER", "4"))
WARMUP_MMS = int(os.environ.get("K_WARMUP", "6"))

CDT = mybir.dt.bfloat16      # compute dtype on device
NP_CDT = ml_dtypes.bfloat16
ODT = mybir.dt.bfloat16      # device output dtype (host upcasts)
NP_ODT = ml_dtypes.bfloat16

_PROG = None
LAST_RESULTS = None          # test harness reads exec_time_ns from here


def _ins(x):
    return getattr(x, "ins", x)


def _build_program():
    """Device program (identical on all 8 cores).

    DRAM inputs are host-packed, partition-major, in consume order:
      wv0 [8, 128, 2048]: batch k = xt_k | w0_k (column blocks of 1024;
                          partition p = K-row k*128+p)
      wv1 [128, 8192]:    slot-1 weight, k-tile k at cols k*1024
      wv2 [128, 8192]:    slot-2 weight, likewise
    Each wave is one large contiguous DMA (8-16KB per-partition lines) so
    data arrives in descriptor order; wv1/wv2 issue is gated on compute
    progress so they never steal bandwidth from the startup ramp.
    """
    nc = bacc.Bacc("TRN2", target_bir_lowering=False, debug=False,
                   num_devices=NCORES)
    wv0_d = nc.dram_tensor("wv0", [KT, 128, 2 * 1024], CDT,
                           kind="ExternalInput")
    wv1_d = nc.dram_tensor("wv1", [128, KT * 1024], CDT,
                           kind="ExternalInput")
    wv2_d = nc.dram_tensor("wv2", [128, KT * 1024], CDT,
                           kind="ExternalInput")
    o_d = nc.dram_tensor("o", [TPC, D], ODT, kind="ExternalOutput")

    with tile.TileContext(nc) as tc, ExitStack() as ctx:
        ld_pool = ctx.enter_context(tc.tile_pool(name="ld", bufs=1))
        ps_pool = ctx.enter_context(
            tc.tile_pool(name="ps", bufs=int(os.environ.get("K_PSBUFS", "7")),
                         space=bass.MemorySpace.PSUM))
        out_pool = ctx.enter_context(tc.tile_pool(name="out", bufs=4))
        warm_pool = ctx.enter_context(tc.tile_pool(name="warm", bufs=1))

        # --- PE warmup: junk matmuls so HAM un-throttles while the first
        # real tiles are still in flight.
        if WARMUP_MMS:
            wt = warm_pool.tile([128, 512], CDT, tag="warm_sb")
            nc.gpsimd.memset(wt[:], 0)
            wps_pool = ctx.enter_context(
                tc.tile_pool(name="wps", bufs=1, space=bass.MemorySpace.PSUM))
            wps = wps_pool.tile([128, 512], mybir.dt.float32, tag="warm_ps")
            for _ in range(WARMUP_MMS):
                nc.tensor.matmul(wps[:], wt[:, 0:128], wt[:],
                                 start=True, stop=True)

        # --- Wave-0 batches (one per k-tile: xt_k | w0_k, 0.5MB each),
        # laddered so arrival is progressive and in consume order.
        b_sb = []
        b_dma = []
        for b in range(KT):
            t = ld_pool.tile([128, 2 * 1024], CDT, tag=f"b{b}")
            inst = nc.sync.dma_start(t[:], wv0_d[b])
            if b >= 5:
                add_dep_helper(_ins(inst), _ins(b_dma[b - 5]),
                               sync=True, reason="wave0 ladder")
            b_sb.append(t)
            b_dma.append(inst)
        # Late weight waves, split in two halves (k 0-3 / k 4-7) so the
        # first half's completion unblocks compute without waiting for the
        # whole 2MB + receipt latency.
        HALF = KT // 2 * 1024
        wv1_sb = ld_pool.tile([128, KT * 1024], CDT, tag="wv1")
        wv1_dma = [nc.sync.dma_start(wv1_sb[:, h * HALF:(h + 1) * HALF],
                                     wv1_d[:, h * HALF:(h + 1) * HALF])
                   for h in range(2)]
        wv2_sb = ld_pool.tile([128, KT * 1024], CDT, tag="wv2")
        wv2_dma = [nc.sync.dma_start(wv2_sb[:, h * HALF:(h + 1) * HALF],
                                     wv2_d[:, h * HALF:(h + 1) * HALF])
                   for h in range(2)]

        # Accessors: lhsT [128,128] and rhs [128,512] slices per (k, ...).
        def xt_ap(k, rt):
            return b_sb[k][:, rt * 128:(rt + 1) * 128]

        def w_ap(s, k, oh):
            if s == 0:
                return b_sb[k][:, 1024 + oh * 512: 1024 + (oh + 1) * 512]
            t = wv1_sb if s == 1 else wv2_sb
            return t[:, k * 1024 + oh * 512: k * 1024 + (oh + 1) * 512]

        # --- Compute: per row tile, two 512-wide output halves, each an
        # 8-matmul K-chain into one PSUM bank.
        trigger_mm = {}
        for rt in range(RT):
            s = PATTERN[rt]
            ot = out_pool.tile([128, D], ODT, tag="ot")
            for oh in range(2):
                ps = ps_pool.tile([128, 512], mybir.dt.float32, tag="ps")
                for k in range(KT):
                    mm = nc.tensor.matmul(
                        ps[:],
                        xt_ap(k, rt),
                        w_ap(s, k, oh),
                        start=(k == 0),
                        stop=(k == KT - 1),
                    )
                    if (rt, oh, k) == (0, 0, 0):
                        trigger_mm["wv1a"] = mm
                    if (rt, oh, k) == (0, 0, 4):
                        trigger_mm["wv1b"] = mm
                    if (rt, oh, k) == (3, 0, 0):
                        trigger_mm["wv2a"] = mm
                    if (rt, oh, k) == (3, 0, 4):
                        trigger_mm["wv2b"] = mm
                nc.vector.tensor_copy(ot[:, oh * 512:(oh + 1) * 512], ps[:])
                # store each half as soon as it's copied (scalar HWDGE
                # ring, so stores never stall behind load waits)
                nc.scalar.dma_start(
                    o_d[rt * 128:(rt + 1) * 128, oh * 512:(oh + 1) * 512],
                    ot[:, oh * 512:(oh + 1) * 512])

        # Gate late weight waves on compute progress (not on DMA chains):
        # they start streaming while wave-0's tail is in flight but can't
        # front-run the whole ramp.
        add_dep_helper(_ins(wv1_dma[0]), _ins(trigger_mm["wv1a"]),
                       sync=True, reason="wv1a after slot0 start")
        add_dep_helper(_ins(wv1_dma[1]), _ins(trigger_mm["wv1b"]),
                       sync=True, reason="wv1b after slot0 mid")
        add_dep_helper(_ins(wv2_dma[0]), _ins(trigger_mm["wv2a"]),
                       sync=True, reason="wv2a after slot1 start")
        add_dep_helper(_ins(wv2_dma[1]), _ins(trigger_mm["wv2b"]),
                       sync=True, reason="wv2b after slot1 mid")

    nc.compile()
    return nc


def _build_program_raw():
    """Raw (no-Tile) variant of the same dataflow: manual semaphores,
    straight-line per-engine programs. Skips Tile's startup/teardown
    barrier machinery (~8us of the Tile version's span)."""
    nc = bacc.Bacc("TRN2", target_bir_lowering=False, debug=False,
                   num_devices=NCORES)
    wv0_d = nc.dram_tensor("wv0", [KT, 128, 2 * 1024], CDT,
                           kind="ExternalInput")
    wv1_d = nc.dram_tensor("wv1", [128, KT * 1024], CDT,
                           kind="ExternalInput")
    wv2_d = nc.dram_tensor("wv2", [128, KT * 1024], CDT,
                           kind="ExternalInput")
    o_d = nc.dram_tensor("o", [TPC, D], ODT, kind="ExternalOutput")

    b_sb = [nc.alloc_sbuf_tensor(f"b{k}", [128, 2 * 1024], CDT).ap()
            for k in range(KT)]
    wv1_sb = nc.alloc_sbuf_tensor("wv1s", [128, KT * 1024], CDT).ap()
    wv2_sb = nc.alloc_sbuf_tensor("wv2s", [128, KT * 1024], CDT).ap()
    ot_sb = [nc.alloc_sbuf_tensor(f"ot{i}", [128, D], ODT).ap()
             for i in range(4)]
    warm_sb = nc.alloc_sbuf_tensor("warm", [128, 512], CDT).ap()
    psum = [nc.alloc_psum_tensor(f"ps{i}", [128, 512], mybir.dt.float32).ap()
            for i in range(7)]
    wps = nc.alloc_psum_tensor("wps", [128, 512], mybir.dt.float32).ap()

    s_b = [nc.alloc_semaphore(f"sb{k}") for k in range(KT)]
    s_w1a = nc.alloc_semaphore("sw1a")
    s_w1b = nc.alloc_semaphore("sw1b")
    s_w2a = nc.alloc_semaphore("sw2a")
    s_w2b = nc.alloc_semaphore("sw2b")
    mmk_sem = nc.alloc_semaphore("mmk")    # trigger points for late waves
    mm_sem = nc.alloc_semaphore("mm")      # chain completions
    cp_sem = nc.alloc_semaphore("cp")      # copy completions
    z_sem = nc.alloc_semaphore("z")        # warmup tile zeroed
    # per (ot buffer, half) store-completion sems for staging reuse
    s_st = [[nc.alloc_semaphore(f"st{i}_{j}") for j in range(2)]
            for i in range(4)]

    chains = [(rt, oh) for rt in range(RT) for oh in range(2)]
    HALF = KT // 2 * 1024

    def xt_ap(k, rt):
        return b_sb[k][:, rt * 128:(rt + 1) * 128]

    def w_ap(s, k, oh):
        if s == 0:
            return b_sb[k][:, 1024 + oh * 512: 1024 + (oh + 1) * 512]
        t = wv1_sb if s == 1 else wv2_sb
        return t[:, k * 1024 + oh * 512: k * 1024 + (oh + 1) * 512]

    with nc.Block() as block:

        @block.sync
        def _(sync):
            # wave-0 loads, depth-5 ladder
            for k in range(KT):
                if k >= 5:
                    sync.wait_ge(s_b[k - 5], 16)
                sync.dma_start(b_sb[k][:], wv0_d[k]).then_inc(s_b[k], 16)
            # late weight waves, gated on compute progress
            for trig, (dst, src, lo, hi, sem) in enumerate([
                    (wv1_sb, wv1_d, 0, HALF, s_w1a),
                    (wv1_sb, wv1_d, HALF, 2 * HALF, s_w1b),
                    (wv2_sb, wv2_d, 0, HALF, s_w2a),
                    (wv2_sb, wv2_d, HALF, 2 * HALF, s_w2b)]):
                sync.wait_ge(mmk_sem, trig + 1)
                sync.dma_start(dst[:, lo:hi], src[:, lo:hi]).then_inc(sem, 16)
            # quiesce: all stores complete (each buffer-half stored twice)
            for i in range(4):
                for j in range(2):
                    sync.wait_ge(s_st[i][j], 32)

        @block.gpsimd
        def _(g):
            g.memset(warm_sb[:], 0).then_inc(z_sem)

        # PSUM bank per chain: slot 0 -> 0-5, slot 1 -> 6,7,0-3 (7 = the
        # warmup bank, free after warmup), slot 2 -> 4-7. Reused banks
        # wait for the prior chain's PSUM->SBUF copy.
        all_banks = psum + [wps]
        bank_of = [0, 1, 2, 3, 4, 5, 6, 7, 0, 1, 2, 3, 4, 5, 6, 7]
        reuse_cp = {8: 1, 9: 2, 10: 3, 11: 4, 12: 5, 13: 6, 14: 7, 15: 8}
        slot_chains = [[ci for ci, (rt, _) in enumerate(chains)
                        if PATTERN[rt] == s] for s in range(NSLOTS)]

        @block.tensor
        def _(te):
            for i in range(WARMUP_MMS):
                mm = te.matmul(wps[:], warm_sb[:, 0:128], warm_sb[:],
                               start=True, stop=True)
                if i == 0:
                    mm._wait_ge(z_sem, 1)
            for s in range(NSLOTS):
                # slot 0 is DMA-paced: iterate k-major so every open
                # chain advances as each batch lands (PE is in-order).
                # slots 1-2 have resident data: iterate chain-major so
                # stop-MMs (and thus copies, stores, bank releases)
                # stagger instead of bursting at the slot boundary.
                order = [(k, ci) for k in range(KT)
                         for ci in slot_chains[s]]
                for k, ci in order:
                    rt, oh = chains[ci]
                    if k == 0 and ci in reuse_cp:
                        te.wait_ge(cp_sem, reuse_cp[ci])
                    mm = te.matmul(all_banks[bank_of[ci]][:],
                                   xt_ap(k, rt), w_ap(s, k, oh),
                                   start=(k == 0), stop=(k == KT - 1))
                    if ci == slot_chains[s][0]:
                        if s == 0:
                            mm._wait_ge(s_b[k], 16)
                        elif s == 1:
                            mm._wait_ge(s_w1a if k < 4 else s_w1b, 16)
                        else:
                            mm._wait_ge(s_w2a if k < 4 else s_w2b, 16)
                    if (rt, oh, k) in ((0, 0, 0), (0, 0, 4),
                                       (3, 0, 0), (3, 0, 4)):
                        mm.then_inc(mmk_sem)
                    if k == KT - 1:
                        mm.then_inc(mm_sem)

        @block.vector
        def _(ve):
            for ci, (rt, oh) in enumerate(chains):
                if rt >= 4:          # ot buffer-half recycled after store
                    ve.wait_ge(s_st[rt % 4][oh], 16)
                cp = ve.tensor_copy(
                    ot_sb[rt % 4][:, oh * 512:(oh + 1) * 512],
                    all_banks[bank_of[ci]][:])
                cp._wait_ge(mm_sem, ci + 1)
                cp.then_inc(cp_sem)

        @block.scalar
        def _(sc):
            for ci, (rt, oh) in enumerate(chains):
                sc.wait_ge(cp_sem, ci + 1)
                sc.dma_start(
                    o_d[rt * 128:(rt + 1) * 128, oh * 512:(oh + 1) * 512],
                    ot_sb[rt % 4][:, oh * 512:(oh + 1) * 512],
                ).then_inc(s_st[rt % 4][oh], 16)

    nc.compile()
    return nc


def _get_program():
    global _PROG
    if _PROG is None:
        if os.environ.get("K_RAW", "0") == "1":
            _PROG = _build_program_raw()
        else:
            _PROG = _build_program()
    return _PROG


def _solve_parts(tiles_per_expert):
    """Decompose per-expert tile counts into 16 parts of 3 tiles and 8
    parts of 2 tiles. Returns (threes, twos) as lists of expert ids, or
    None if infeasible."""
    t = list(tiles_per_expert)
    f = [c % 2 for c in t]              # number of 3-parts per expert
    if any(3 * f[g] > t[g] for g in range(len(t))):
        return None
    h = [(t[g] - 3 * f[g]) // 2 for g in range(len(t))]
    # each f+=2 converts three 2-parts into two 3-parts
    while sum(h) > 8:
        g = max(range(len(t)), key=lambda i: h[i])
        if h[g] < 3:
            return None
        f[g] += 2
        h[g] -= 3
    if sum(h) != 8 or sum(f) != 16:
        return None
    threes, twos = [], []
    for g in range(len(t)):
        threes += [g] * f[g]
        twos += [g] * h[g]
    return threes, twos


def _numpy_fallback(hidden_states, weight, counts):
    out = np.empty((hidden_states.shape[0], weight.shape[2]), np.float32)
    start = 0
    for g in range(weight.shape[0]):
        end = start + int(counts[g])
        out[start:end] = hidden_states[start:end].astype(np.float32) @ \
            weight[g].astype(np.float32)
        start = end
    return out


def kernel(hidden_states, weight, tokens_per_expert):
    counts = np.asarray(tokens_per_expert).astype(np.int64)
    out_dtype = hidden_states.dtype

    ok = (hidden_states.shape == (T, D) and weight.shape == (G, D, D)
          and counts.shape == (G,) and counts.sum() == T
          and np.all(counts % 128 == 0) and np.all(counts >= 0))
    parts = _solve_parts(counts // 128) if ok else None
    if parts is None:
        return _numpy_fallback(hidden_states, weight, counts).astype(out_dtype)
    threes, twos = parts

    # Global preprocessing: transpose+cast activations once, cast weights.
    ht = np.ascontiguousarray(
        hidden_states.astype(NP_CDT).T)          # [D, T] bf16
    wc = weight.astype(NP_CDT)                    # [G, D, D] bf16

    # Per-expert global row offsets; consume tiles in order.
    expert_row = dict(
        (g, int(o)) for g, o in enumerate(np.concatenate(
            [[0], np.cumsum(counts)[:-1]])))

    in_maps = []
    core_rows = []       # per core: list of (global_row_start, n_rows)
    for c in range(NCORES):
        part_list = [(threes[2 * c], 3 * 128), (threes[2 * c + 1], 3 * 128),
                     (twos[c], 2 * 128)]
        spans = []
        for g, nrows in part_list:
            r0 = expert_row[g]
            expert_row[g] = r0 + nrows
            spans.append((r0, nrows))
        core_rows.append(spans)
        # xt_c: [D, TPC] activations (pre-transposed); k-tile k = rows
        # k*128..k*128+127.
        xt_c = np.concatenate(
            [ht[:, r0:r0 + n] for r0, n in spans], axis=1)
        w_slots = [wc[g] for g, _ in part_list]   # 3 x [D, D]

        # wv0 [KT, 128, 2048]: batch k packs k-tile k of xt and w0,
        # partition-major: wv0[k, p] = xt[k*128+p, :] | w0[k*128+p, :]
        xt_k = xt_c.reshape(KT, 128, TPC)
        w0_k = w_slots[0].reshape(KT, 128, D)
        wv0 = np.empty((KT, 128, 2 * 1024), dtype=NP_CDT)
        for b in range(KT):
            wv0[b, :, 0:1024] = xt_k[b]
            wv0[b, :, 1024:2048] = w0_k[b]
        # wv1/wv2 [128, 8192]: row p = concat_k W[k*128+p, :]
        wv1 = np.ascontiguousarray(
            w_slots[1].reshape(KT, 128, D).transpose(1, 0, 2).reshape(
                128, KT * D))
        wv2 = np.ascontiguousarray(
            w_slots[2].reshape(KT, 128, D).transpose(1, 0, 2).reshape(
                128, KT * D))
        in_maps.append({"wv0": wv0, "wv1": wv1, "wv2": wv2})

    nc = _get_program()
    global LAST_RESULTS
    LAST_RESULTS = run_bass_kernel_spmd(nc, in_maps, list(range(NCORES)))

    out = np.empty((T, D), np.float32)
    for c in range(NCORES):
        o_c = np.asarray(LAST_RESULTS.results[c]["o"]).astype(np.float32)
        r = 0
        for r0, n in core_rows[c]:
            out[r0:r0 + n] = o_c[r:r + n]
            r += n
    return out.astype(out_dtype, copy=False)
